# revision 31
# baseline (speedup 1.0000x reference)
"""2-layer GAT (PyG-style) on 8 Trainium2 NeuronCores via Bass/Tile.

Self-contained: kernel(**inputs) -> [100000, 40] float32.

Design (v2, "slot-rounds"):
  Nodes sorted by in-degree, grouped into 784 tiles of 128; tiles dealt
  round-robin to 8 cores (98 dst-tiles each, degree-homogeneous so all cores
  share one per-tile round count K[t]). Edges of dst p in tile t occupy
  "slot rounds" k=0..deg-1 at partition p; padding rounds point at a sentinel
  table row whose attention logit is -100 (exp ~= 0 even after leaky-relu).

  One SPMD program per core:
    dense:  h|a_src|a_dst = xT_c @ [W1|Ws1|Wd1] for the core's own 98 tiles
            -> hloc [NPC,136] fp16, adt1 [NPC,8] f32
    AG1:    AllGather hloc -> table1 [NPAD,136] (+sentinel row)
    L1 edge: per tile: K indirect row-gathers (128 rows each) from table1,
            e = a_src[src]+a_dst[dst], leaky-relu, exp (no max-sub needed;
            |e| small), msg = h*ex, aggregation = K+1 identity matmuls
            accumulating [h*ex | ex] into PSUM (self-loop via the tile's own
            sequential rows), normalize, +b1, ELU -> g; a2 scalars by
            linearity (g.(W2@att2)) -> gloc [NPC,136] fp16, adt2 [NPC,1]
    AG2:    AllGather gloc -> table2 (+sentinel)
    L2 edge: same gather structure in g-space (1 head); aggregation
            transposed (lhsT=g*ex chunks, rhs=I) -> aggT; out = aggT@W2/s+b2

Falls back to a pure-numpy forward if the device path fails.
"""
import sys
sys.path.insert(0, "/opt/trn_rl_repo")
sys.path.insert(0, "/root/.axon_site")
import numpy as np

N_CORES = 8
TPC = 98
NCLASS = 40
NEG = 0.2
ROW = 136

_CACHE = {}


# ----------------------------------------------------------------- numpy ref

def _np_forward(x, edge_index, W1, a_s1, a_d1, b1, W2, a_s2, a_d2, b2):
    N = x.shape[0]
    src = np.concatenate([np.asarray(edge_index[0], np.int64), np.arange(N)])
    dst = np.concatenate([np.asarray(edge_index[1], np.int64), np.arange(N)])
    o = np.argsort(dst, kind="stable")
    src, dst = src[o], dst[o]
    starts = np.searchsorted(dst, np.arange(N))

    def gat(xx, W, a_s, a_d, bb, concat):
        H, C = a_s.shape
        h = (xx @ np.asarray(W, xx.dtype)).reshape(-1, H, C)
        asr = np.einsum("nhc,hc->nh", h, np.asarray(a_s, xx.dtype))
        ads = np.einsum("nhc,hc->nh", h, np.asarray(a_d, xx.dtype))
        e = asr[src] + ads[dst]
        e = np.where(e >= 0, e, NEG * e)
        ex = np.exp(e)
        s = np.add.reduceat(ex, starts, axis=0)
        alpha = ex / s[dst]
        msg = (h[src] * alpha[:, :, None]).reshape(len(src), -1)
        out = np.add.reduceat(msg, starts, axis=0).reshape(N, H, C)
        out = out.reshape(N, H * C) if concat else out.mean(axis=1)
        return out + np.asarray(bb, xx.dtype)

    h = gat(x.astype(np.float64), W1, a_s1, a_d1, b1, True)
    h = np.where(h > 0, h, np.exp(np.minimum(h, 0)) - 1.0)
    out = gat(h, W2, a_s2, a_d2, b2, False)
    return out.astype(np.float32)


# ----------------------------------------------------------------- host prep

def _prep_host(x, edge_index, n_cores=N_CORES, tpc=TPC):
    N, FIN = x.shape
    npc = tpc * 128
    npad = n_cores * npc
    assert npad >= N
    src = np.asarray(edge_index[0], np.int64)
    dst = np.asarray(edge_index[1], np.int64)
    deg = np.bincount(dst, minlength=npad)

    order = np.argsort(-deg, kind="stable")
    node_of_row = np.empty(npad, np.int64)
    ntiles = npad // 128
    k_arr = np.arange(ntiles)
    c_of_tile = k_arr % n_cores
    t_of_tile = k_arr // n_cores
    for k in range(ntiles):
        c, t = int(c_of_tile[k]), int(t_of_tile[k])
        node_of_row[c * npc + t * 128:(c * npc + t * 128) + 128] = \
            order[k * 128:(k + 1) * 128]
    row_of_node = np.empty(npad, np.int64)
    row_of_node[node_of_row] = np.arange(npad)

    # shared per-tile round counts: max degree within rank window of 8 tiles
    K = np.maximum(1, deg[order[np.arange(tpc) * (n_cores * 128)]]).astype(
        np.int64)
    Kcum = np.concatenate([[0], np.cumsum(K)]).astype(np.int64)
    KTOT = int(K[-1] + Kcum[-2]) if tpc > 1 else int(K[0])
    KTOT = int(Kcum[-1])

    # per-edge slot assignment (vectorized)
    erow = row_of_node[dst]
    eord = np.argsort(erow, kind="stable")
    src_s = src[eord]
    erow_s = erow[eord]
    starts = np.searchsorted(erow_s, np.arange(npad + 1))
    kidx = np.arange(len(src_s)) - np.repeat(starts[:-1],
                                             np.diff(starts))
    ec = erow_s // npc
    et = (erow_s % npc) // 128
    ep = erow_s % 128
    offs = np.full((n_cores, 128, KTOT), npad, np.int32)
    offs[ec, ep, Kcum[et] + kidx] = row_of_node[src_s].astype(np.int32)

    xT = np.zeros((n_cores, 128, npc), dtype=np.float16)
    xpad = np.zeros((npad, FIN), np.float32)
    xpad[:N] = np.asarray(x, np.float32)
    for c in range(n_cores):
        xT[c] = xpad[node_of_row[c * npc:(c + 1) * npc]].T.astype(
            np.float16)

    meta = dict(K=K.tolist(), Kcum=Kcum.tolist(), KTOT=KTOT,
                node_of_row=node_of_row, N=N, n_cores=n_cores, tpc=tpc,
                npc=npc, npad=npad)
    return offs, xT, meta


def _prep_weights(W1, a_s1, a_d1, b1, W2, a_s2, a_d2, b2):
    W1 = np.asarray(W1, np.float32)
    H, C = np.asarray(a_s1).shape
    FIN = W1.shape[0]
    Ws1 = np.zeros((FIN, H), np.float32)
    Wd1 = np.zeros((FIN, H), np.float32)
    for h in range(H):
        Ws1[:, h] = W1[:, h * C:(h + 1) * C] @ np.asarray(a_s1, np.float32)[h]
        Wd1[:, h] = W1[:, h * C:(h + 1) * C] @ np.asarray(a_d1, np.float32)[h]
    W1cat = np.concatenate([W1, Ws1, Wd1], axis=1)
    W2 = np.asarray(W2, np.float32)
    HID = W1.shape[1]
    sent = np.zeros((1, ROW), np.float32)
    sent[0, HID:HID + 8] = -100.0
    return dict(
        W1cat=W1cat.astype(np.float16),
        identb=np.eye(128, dtype=np.float32).astype(np.float16),
        W2s=W2,
        b1row=np.asarray(b1, np.float32).reshape(1, HID),
        b2row=np.asarray(b2, np.float32).reshape(1, NCLASS),
        ws2row=(W2 @ np.asarray(a_s2, np.float32)[0]).reshape(1, HID),
        wd2row=(W2 @ np.asarray(a_d2, np.float32)[0]).reshape(1, HID),
        ones_row=np.ones((1, 128), np.float32),
        sent=sent.astype(np.float16),
    )


# ----------------------------------------------------------------- builder

def _build(meta, timeline=False, phases=4, tlim=None, strip=127):
    from concourse import bass, bacc, mybir, tile
    F32, BF16, I32 = mybir.dt.float32, mybir.dt.float16, mybir.dt.int32
    MULT, ADD, MAXOP, SUB = (mybir.AluOpType.mult, mybir.AluOpType.add,
                             mybir.AluOpType.max, mybir.AluOpType.subtract)
    EXPF = mybir.ActivationFunctionType.Exp
    K, Kcum, KTOT = meta["K"], meta["Kcum"], meta["KTOT"]
    tpc, npc, npad = meta["tpc"], meta["npc"], meta["npad"]
    tlim = tpc if tlim is None else tlim
    n_cores = 1 if timeline else meta["n_cores"]
    ncol = meta["n_cores"]

    nc = bacc.Bacc("TRN2", target_bir_lowering=False, debug=False,
                   num_devices=n_cores)
    xT = nc.dram_tensor("xT", [128, npc], BF16, kind="ExternalInput")
    offs = nc.dram_tensor("offs", [128, KTOT], I32, kind="ExternalInput")
    W1cat = nc.dram_tensor("W1cat", [128, 144], BF16, kind="ExternalInput")
    identb = nc.dram_tensor("identb", [128, 128], BF16, kind="ExternalInput")
    W2s = nc.dram_tensor("W2s", [128, NCLASS], F32, kind="ExternalInput")
    sentd = nc.dram_tensor("sent", [1, ROW], BF16, kind="ExternalInput")
    H = {}
    for nm in ("b1row", "ws2row", "wd2row", "ones_row"):
        H[nm] = nc.dram_tensor(nm, [1, 128], F32, kind="ExternalInput")
    H["b2row"] = nc.dram_tensor("b2row", [1, NCLASS], F32,
                                kind="ExternalInput")
    hloc = nc.dram_tensor("hloc", [npc, ROW], BF16)
    adt1 = nc.dram_tensor("adt1", [npc, 8], F32)
    table1 = nc.dram_tensor("table1", [npad + 1, ROW], BF16)
    gloc = nc.dram_tensor("gloc", [npc, ROW], BF16)
    adt2 = nc.dram_tensor("adt2", [npc, 1], F32)
    table2 = nc.dram_tensor("table2", [npad + 1, ROW], BF16)
    out2 = nc.dram_tensor("out2", [npc, NCLASS], F32, kind="ExternalOutput")
    groups = [list(range(ncol))]

    def allgather(tc, src_t, dst_t):
        if timeline:
            for j in range(ncol):
                nc.sync.dma_start(out=dst_t[j * npc:(j + 1) * npc, :],
                                  in_=src_t[0:npc, :])
        else:
            nc.gpsimd.collective_compute(
                "AllGather", mybir.AluOpType.bypass, groups,
                ins=[src_t[0:npc, :]], outs=[dst_t[0:npad, :]])

    with tile.TileContext(nc) as tc:
        with (
            tc.tile_pool(name="const", bufs=1) as constp,
            tc.tile_pool(name="io", bufs=4) as io,
            tc.tile_pool(name="gp", bufs=3) as gp,
            tc.tile_pool(name="est", bufs=4) as est,
            tc.tile_pool(name="stg", bufs=4) as stg,
        ):
            # ---- constants
            ident = constp.tile([128, 128], BF16, name="ident")
            nc.sync.dma_start(out=ident[:], in_=identb[:, :])
            w1c = constp.tile([128, 144], BF16, name="w1c")
            nc.sync.dma_start(out=w1c[:], in_=W1cat[:, :])
            w2t = constp.tile([128, NCLASS], F32, name="w2t")
            nc.sync.dma_start(out=w2t[:], in_=W2s[:, :])
            onesr = constp.tile([1, 128], F32, name="onesr")
            nc.sync.dma_start(out=onesr[:], in_=H["ones_row"].ap())
            sent_t = constp.tile([1, ROW], BF16, name="sent_t")
            nc.sync.dma_start(out=sent_t[:], in_=sentd[:, :])
            rows = constp.tile([1, 384 + NCLASS], F32, name="rows")
            for i, nm in enumerate(["b1row", "ws2row", "wd2row"]):
                nc.sync.dma_start(out=rows[:, i * 128:(i + 1) * 128],
                                  in_=H[nm].ap())
            nc.sync.dma_start(out=rows[:, 384:384 + NCLASS],
                              in_=H["b2row"].ap())
            C = {}
            with tc.tile_pool(name="psC", bufs=2, space="PSUM") as psC:
                for nm, lo, n in (("b1F", 0, 128), ("ws2F", 128, 128),
                                  ("wd2F", 256, 128), ("b2F", 384, NCLASS)):
                    t_ = constp.tile([128, n], F32, name=nm)
                    prt = psC.tile([128, 128], F32, tag="rep", name="prt")
                    nc.tensor.matmul(prt[:, 0:n], lhsT=onesr[:],
                                     rhs=rows[:, lo:lo + n],
                                     start=True, stop=True)
                    nc.vector.tensor_copy(t_[:], prt[:, 0:n])
                    C[nm] = t_

            # ---- dense phase (own tiles only)
            with (
                tc.tile_pool(name="xin", bufs=1) as xin,
                tc.tile_pool(name="psD", bufs=4, space="PSUM") as psD,
            ):
                xfull = xin.tile([128, npc], BF16, name="xfull")
                nc.sync.dma_start(out=xfull[:], in_=xT[:, :])
                DB = 7  # tiles per batched write (98 = 14 x 7)
                for t0 in range(0, tpc, DB):
                    nb = min(DB, tpc - t0)
                    hst = stg.tile([128, DB * ROW], BF16, tag="hst",
                                   name="hst")
                    ast = stg.tile([128, DB * 8], F32, tag="ast", name="ast")
                    for j in range(nb):
                        t = t0 + j
                        ph = psD.tile([128, 144], F32, tag="ph", name="ph")
                        nc.tensor.matmul(ph[:], lhsT=xfull[:, t*128:(t+1)*128],
                                         rhs=w1c[:], start=True, stop=True)
                        if t % 2 == 0:
                            nc.vector.tensor_copy(hst[:, j*ROW:(j+1)*ROW],
                                                  ph[:, 0:ROW])
                        else:
                            nc.scalar.copy(hst[:, j*ROW:(j+1)*ROW],
                                           ph[:, 0:ROW])
                        nc.vector.tensor_copy(ast[:, j*8:(j+1)*8],
                                              ph[:, 136:144])
                    nc.sync.dma_start(
                        out=hloc[t0*128:(t0+nb)*128, :].rearrange(
                            "(t p) f -> p t f", p=128),
                        in_=hst[:, 0:nb*ROW].rearrange("p (t f) -> p t f",
                                                       f=ROW))
                    nc.sync.dma_start(
                        out=adt1[t0*128:(t0+nb)*128, :].rearrange(
                            "(t p) f -> p t f", p=128),
                        in_=ast[:, 0:nb*8].rearrange("p (t f) -> p t f", f=8))

            allgather(tc, hloc, table1)
            nc.sync.dma_start(out=table1[npad:npad + 1, :], in_=sent_t[:])

            # ---- layer-1 edge phase
            with tc.tile_pool(name="psA", bufs=2, space="PSUM") as psA:
                for t in range(tlim if phases >= 2 else 0):
                    Kt = K[t]
                    c0 = Kcum[t]
                    ot = io.tile([128, Kt], I32, tag="ot", name="ot")
                    nc.sync.dma_start(out=ot[:], in_=offs[:, c0:c0 + Kt])
                    adt = io.tile([128, 8], F32, tag="adt", name="adt")
                    nc.sync.dma_start(out=adt[:], in_=adt1[t*128:(t+1)*128, :])
                    hD = io.tile([128, ROW], BF16, tag="hD", name="hD")
                    nc.sync.dma_start(out=hD[:], in_=hloc[t*128:(t+1)*128, :])
                    G = gp.tile([128, Kt * ROW], BF16, tag="G", name="G")
                    if strip & 1:
                        for k in range(Kt):
                            nc.gpsimd.indirect_dma_start(
                                out=G[:, k*ROW:(k+1)*ROW], out_offset=None,
                                in_=table1[:, :],
                                in_offset=bass.IndirectOffsetOnAxis(
                                    ap=ot[:, k:k + 1], axis=0))
                    if strip != 127:
                        gc0 = stg.tile([128, ROW], BF16, tag="gcat", name="gc0")
                        if strip & 1:
                            nc.vector.tensor_copy(gc0[:], G[:, 0:ROW])
                        else:
                            nc.vector.tensor_copy(gc0[:], hD[:])
                        nc.sync.dma_start(out=gloc[t*128:(t+1)*128, :],
                                          in_=gc0[:])
                    # e = a_src + a_dst ; leaky ; exp  (views ordered [p,h,k])
                    Gas = G[:].rearrange("p (k f) -> p f k", k=Kt)[:, 128:136, :]
                    E = est.tile([128, Kt * 8], F32, tag="E", name="E")
                    Ev = E[:].rearrange("p (h k) -> p h k", h=8)
                    if strip & 2:
                        nc.vector.tensor_tensor(
                            Ev, Gas, adt[:].to_broadcast([128, 8, Kt]), ADD)
                        E2 = est.tile([128, Kt * 8], F32, tag="E2", name="E2")
                        nc.vector.tensor_scalar_mul(E2[:], E[:], NEG)
                        nc.vector.tensor_tensor(E[:], E[:], E2[:], MAXOP)
                        eS = est.tile([128, 8], F32, tag="eS", name="eS")
                        nc.vector.tensor_tensor(eS[:], hD[:, 128:136], adt[:],
                                                ADD)
                        eS2 = est.tile([128, 8], F32, tag="eS2", name="eS2")
                        nc.vector.tensor_scalar_mul(eS2[:], eS[:], NEG)
                        nc.vector.tensor_tensor(eS[:], eS[:], eS2[:], MAXOP)
                    if strip & 4:
                        nc.scalar.activation(Gas, Ev, EXPF)
                        nc.scalar.activation(hD[:, 128:136], eS[:], EXPF)
                    # msg = h * ex
                    if strip & 8:
                        Gh = G[:].rearrange("p (k f) -> p k f", k=Kt)[:, :, 0:128] \
                            .rearrange("p k (h c) -> p k h c", c=16)
                        Gex = G[:].rearrange("p (k f) -> p k f",
                                             k=Kt)[:, :, 128:136]
                        nc.vector.tensor_tensor(
                            Gh, Gh, Gex.to_broadcast([128, Kt, 8, 16]), MULT)
                        hDh = hD[:, 0:128].rearrange("p (h c) -> p h c", c=16)
                        nc.vector.tensor_tensor(
                            hDh, hDh,
                            hD[:, 128:136].to_broadcast([128, 8, 16]), MULT)
                    if not (strip & 16):
                        continue
                    # aggregate: K+1 identity matmuls
                    ACC = psA.tile([128, ROW], F32, tag="ACC", name="ACC")
                    for k in range(Kt):
                        nc.tensor.matmul(ACC[:], lhsT=ident[:],
                                         rhs=G[:, k*ROW:(k+1)*ROW],
                                         start=(k == 0), stop=False)
                    nc.tensor.matmul(ACC[:], lhsT=ident[:], rhs=hD[:],
                                     start=False, stop=True)
                    if not (strip & 32):
                        gc1 = stg.tile([128, ROW], F32, tag="g1", name="gc1")
                        nc.vector.tensor_copy(gc1[:], ACC[:])
                        nc.sync.dma_start(out=gloc[t*128:(t+1)*128, :],
                                          in_=gc1[:].bitcast(BF16)[:, 0:ROW])
                        continue
                    # close: normalize, +b1, ELU, a2 scalars
                    rcp = est.tile([128, 8], F32, tag="rcp", name="rcp")
                    nc.vector.reciprocal(rcp[:], ACC[:, 128:136])
                    g1 = stg.tile([128, 128], F32, tag="g1", name="g1")
                    g1v = g1[:].rearrange("p (h c) -> p h c", c=16)
                    nc.vector.tensor_tensor(
                        g1v, ACC[:, 0:128].rearrange("p (h c) -> p h c", c=16),
                        rcp[:].to_broadcast([128, 8, 16]), MULT)
                    nc.vector.tensor_tensor(g1[:], g1[:], C["b1F"][:], ADD)
                    mt = est.tile([128, 128], F32, tag="mt", name="mt")
                    nc.vector.tensor_scalar_min(mt[:], g1[:], 0.0)
                    emt = est.tile([128, 128], F32, tag="emt", name="emt")
                    nc.scalar.activation(emt[:], mt[:], EXPF)
                    nc.vector.tensor_scalar(g1[:], g1[:], 0.0, 1.0, MAXOP, SUB)
                    nc.vector.tensor_tensor(g1[:], g1[:], emt[:], ADD)
                    gcat = stg.tile([128, ROW], BF16, tag="gcat", name="gcat")
                    nc.vector.tensor_copy(gcat[:, 0:128], g1[:])
                    nc.vector.memset(gcat[:, 129:136], 0.0)
                    if strip & 64:
                        dmy = est.tile([128, 128], F32, tag="dmy", name="dmy")
                        a2s = est.tile([128, 1], F32, tag="a2s", name="a2s")
                        nc.vector.tensor_tensor(dmy[:], g1[:], C["ws2F"][:],
                                                MULT)
                        nc.vector.tensor_reduce(a2s[:], dmy[:],
                                                mybir.AxisListType.X, ADD)
                        a2d = stg.tile([128, 1], F32, tag="a2d", name="a2d")
                        nc.vector.tensor_tensor(dmy[:], g1[:], C["wd2F"][:],
                                                MULT)
                        nc.vector.tensor_reduce(a2d[:], dmy[:],
                                                mybir.AxisListType.X, ADD)
                        nc.vector.tensor_copy(gcat[:, 128:129], a2s[:])
                        nc.sync.dma_start(out=adt2[t*128:(t+1)*128, :],
                                          in_=a2d[:])
                    else:
                        nc.vector.memset(gcat[:, 128:129], 0.0)
                    nc.sync.dma_start(out=gloc[t*128:(t+1)*128, :],
                                      in_=gcat[:])

            if phases >= 3:
                allgather(tc, gloc, table2)
                nc.sync.dma_start(out=table2[npad:npad + 1, :], in_=sent_t[:])
            if phases < 4:
                for t in range(tpc):
                    nc.sync.dma_start(out=out2[t*128:(t+1)*128, :],
                                      in_=C["b2F"][:])

            # ---- layer-2 edge phase
            with (
                tc.tile_pool(name="psT", bufs=2, space="PSUM") as psT,
                tc.tile_pool(name="psO", bufs=2, space="PSUM") as psO,
            ):
                for t in range(tlim if phases >= 4 else 0):
                    Kt = K[t]
                    c0 = Kcum[t]
                    ot2 = io.tile([128, Kt], I32, tag="ot", name="ot2")
                    nc.sync.dma_start(out=ot2[:], in_=offs[:, c0:c0 + Kt])
                    a2t = io.tile([128, 1], F32, tag="a2t", name="a2t")
                    nc.sync.dma_start(out=a2t[:], in_=adt2[t*128:(t+1)*128, :])
                    gD = io.tile([128, ROW], BF16, tag="hD", name="gD")
                    nc.sync.dma_start(out=gD[:], in_=gloc[t*128:(t+1)*128, :])
                    G2 = gp.tile([128, Kt * ROW], BF16, tag="G", name="G2")
                    for k in range(Kt):
                        nc.gpsimd.indirect_dma_start(
                            out=G2[:, k*ROW:(k+1)*ROW], out_offset=None,
                            in_=table2[:, :],
                            in_offset=bass.IndirectOffsetOnAxis(
                                ap=ot2[:, k:k + 1], axis=0))
                    Gas2 = G2[:].rearrange("p (k f) -> p f k", k=Kt)[:, 128:129, :]
                    E2a = est.tile([128, Kt], F32, tag="E", name="E2a")
                    E2v = E2a[:].rearrange("p (h k) -> p h k", h=1)
                    nc.vector.tensor_tensor(E2v, Gas2,
                                            a2t[:].to_broadcast([128, 1, Kt]),
                                            ADD)
                    E2b = est.tile([128, Kt], F32, tag="E2", name="E2b")
                    nc.vector.tensor_scalar_mul(E2b[:], E2a[:], NEG)
                    nc.vector.tensor_tensor(E2a[:], E2a[:], E2b[:], MAXOP)
                    nc.scalar.activation(Gas2, E2v, EXPF)
                    e2S = est.tile([128, 1], F32, tag="eS", name="e2S")
                    nc.vector.tensor_tensor(e2S[:], gD[:, 128:129], a2t[:],
                                            ADD)
                    e2S2 = est.tile([128, 1], F32, tag="eS2", name="e2S2")
                    nc.vector.tensor_scalar_mul(e2S2[:], e2S[:], NEG)
                    nc.vector.tensor_tensor(e2S[:], e2S[:], e2S2[:], MAXOP)
                    nc.scalar.activation(gD[:, 128:129], e2S[:], EXPF)
                    # s2 = sum_k ex + ex_self
                    s2 = est.tile([128, 1], F32, tag="s2", name="s2")
                    Gex2 = G2[:].rearrange("p (k f) -> p k f", k=Kt)[:, :, 128]
                    nc.vector.tensor_reduce(s2[:], Gex2,
                                            mybir.AxisListType.X, ADD)
                    nc.vector.tensor_tensor(s2[:], s2[:], gD[:, 128:129], ADD)
                    rcp2 = est.tile([128, 1], F32, tag="rcp2", name="rcp2")
                    nc.vector.reciprocal(rcp2[:], s2[:])
                    # msg = g * ex
                    for k in range(Kt):
                        nc.vector.tensor_tensor(
                            G2[:, k*ROW:k*ROW + 128], G2[:, k*ROW:k*ROW + 128],
                            G2[:, k*ROW + 128:k*ROW + 129].to_broadcast(
                                [128, 128]), MULT)
                    nc.vector.tensor_tensor(
                        gD[:, 0:128], gD[:, 0:128],
                        gD[:, 128:129].to_broadcast([128, 128]), MULT)
                    PT = psT.tile([128, 128], F32, tag="PT", name="PT")
                    for k in range(Kt):
                        nc.tensor.matmul(PT[:], lhsT=G2[:, k*ROW:k*ROW + 128],
                                         rhs=ident[:], start=(k == 0),
                                         stop=False)
                    nc.tensor.matmul(PT[:], lhsT=gD[:, 0:128], rhs=ident[:],
                                     start=False, stop=True)
                    aggT = stg.tile([128, 128], F32, tag="aggT", name="aggT")
                    nc.vector.tensor_copy(aggT[:], PT[:])
                    PO = psO.tile([128, NCLASS], F32, tag="PO", name="PO")
                    nc.tensor.matmul(PO[:], lhsT=aggT[:], rhs=w2t[:],
                                     start=True, stop=True)
                    o2 = stg.tile([128, NCLASS], F32, tag="o2", name="o2")
                    nc.vector.tensor_tensor(
                        o2[:], PO[:], rcp2[:].to_broadcast([128, NCLASS]),
                        MULT)
                    nc.vector.tensor_tensor(o2[:], o2[:], C["b2F"][:], ADD)
                    nc.sync.dma_start(out=out2[t*128:(t+1)*128, :], in_=o2[:])
    nc.compile()
    return nc


# ----------------------------------------------------------------- runner

def _device_forward(inputs):
    from concourse import bass_utils
    x = np.asarray(inputs["x"], np.float32)
    ei = np.asarray(inputs["edge_index"])
    offs, xT, meta = _prep_host(x, ei)
    wd = _prep_weights(inputs["W1"], inputs["att_src1"], inputs["att_dst1"],
                       inputs["b1"], inputs["W2"], inputs["att_src2"],
                       inputs["att_dst2"], inputs["b2"])
    if "nc" not in _CACHE:
        _CACHE["nc"] = _build(meta)
    nc = _CACHE["nc"]
    in_maps = [dict(xT=xT[c], offs=offs[c], **wd) for c in range(N_CORES)]
    res = None
    for attempt in range(3):
        try:
            res = bass_utils.run_bass_kernel_spmd(
                nc, in_maps, core_ids=list(range(N_CORES)))
            break
        except Exception as e:
            sys.stderr.write(f"[kernel] exec attempt {attempt} failed "
                             f"({e!r}); retrying\n")
            import time as _time
            _time.sleep(2.0)
    if res is None:
        res = bass_utils.run_bass_kernel_spmd(
            nc, in_maps, core_ids=list(range(N_CORES)))
    npc, N = meta["npc"], meta["N"]
    out_full = np.zeros((meta["npad"], NCLASS), np.float32)
    for c in range(N_CORES):
        out_full[meta["node_of_row"][c * npc:(c + 1) * npc]] = \
            res.results[c]["out2"]
    return out_full[:N]


def kernel(**inputs):
    for attempt in range(2):
        try:
            out = _device_forward(inputs)
            if not np.all(np.isfinite(out)):
                raise RuntimeError("non-finite device output")
            return out
        except Exception as e:
            sys.stderr.write(f"[kernel] device attempt {attempt} failed "
                             f"({e!r})\n")
    sys.stderr.write("[kernel] device path failed; numpy fallback\n")
    return _np_forward(
        np.asarray(inputs["x"], np.float32), inputs["edge_index"],
        inputs["W1"], inputs["att_src1"], inputs["att_dst1"], inputs["b1"],
        inputs["W2"], inputs["att_src2"], inputs["att_dst2"], inputs["b2"])


# revision 37
# speedup vs baseline: 1.0142x; 1.0142x over previous
"""2-layer GAT (PyG-style) on 8 Trainium2 NeuronCores via Bass/Tile.

Self-contained: kernel(**inputs) -> [100000, 40] float32.

Design (v2, "slot-rounds"):
  Nodes sorted by in-degree, grouped into 784 tiles of 128; tiles dealt
  round-robin to 8 cores (98 dst-tiles each, degree-homogeneous so all cores
  share one per-tile round count K[t]). Edges of dst p in tile t occupy
  "slot rounds" k=0..deg-1 at partition p; padding rounds point at a sentinel
  table row whose attention logit is -100 (exp ~= 0 even after leaky-relu).

  One SPMD program per core:
    dense:  h|a_src|a_dst = xT_c @ [W1|Ws1|Wd1] for the core's own 98 tiles
            -> hloc [NPC,136] fp16, adt1 [NPC,8] f32
    AG1:    AllGather hloc -> table1 [NPAD,136] (+sentinel row)
    L1 edge: per tile: K indirect row-gathers (128 rows each) from table1,
            e = a_src[src]+a_dst[dst], leaky-relu, exp (no max-sub needed;
            |e| small), msg = h*ex, aggregation = K+1 identity matmuls
            accumulating [h*ex | ex] into PSUM (self-loop via the tile's own
            sequential rows), normalize, +b1, ELU -> g; a2 scalars by
            linearity (g.(W2@att2)) -> gloc [NPC,136] fp16, adt2 [NPC,1]
    AG2:    AllGather gloc -> table2 (+sentinel)
    L2 edge: same gather structure in g-space (1 head); aggregation
            transposed (lhsT=g*ex chunks, rhs=I) -> aggT; out = aggT@W2/s+b2

Falls back to a pure-numpy forward if the device path fails.
"""
import sys
sys.path.insert(0, "/opt/trn_rl_repo")
sys.path.insert(0, "/root/.axon_site")
import numpy as np

N_CORES = 8
TPC = 98
NCLASS = 40
NEG = 0.2
ROW = 136

_CACHE = {}


# ----------------------------------------------------------------- numpy ref

def _np_forward(x, edge_index, W1, a_s1, a_d1, b1, W2, a_s2, a_d2, b2):
    N = x.shape[0]
    src = np.concatenate([np.asarray(edge_index[0], np.int64), np.arange(N)])
    dst = np.concatenate([np.asarray(edge_index[1], np.int64), np.arange(N)])
    o = np.argsort(dst, kind="stable")
    src, dst = src[o], dst[o]
    starts = np.searchsorted(dst, np.arange(N))

    def gat(xx, W, a_s, a_d, bb, concat):
        H, C = a_s.shape
        h = (xx @ np.asarray(W, xx.dtype)).reshape(-1, H, C)
        asr = np.einsum("nhc,hc->nh", h, np.asarray(a_s, xx.dtype))
        ads = np.einsum("nhc,hc->nh", h, np.asarray(a_d, xx.dtype))
        e = asr[src] + ads[dst]
        e = np.where(e >= 0, e, NEG * e)
        ex = np.exp(e)
        s = np.add.reduceat(ex, starts, axis=0)
        alpha = ex / s[dst]
        msg = (h[src] * alpha[:, :, None]).reshape(len(src), -1)
        out = np.add.reduceat(msg, starts, axis=0).reshape(N, H, C)
        out = out.reshape(N, H * C) if concat else out.mean(axis=1)
        return out + np.asarray(bb, xx.dtype)

    h = gat(x.astype(np.float64), W1, a_s1, a_d1, b1, True)
    h = np.where(h > 0, h, np.exp(np.minimum(h, 0)) - 1.0)
    out = gat(h, W2, a_s2, a_d2, b2, False)
    return out.astype(np.float32)


# ----------------------------------------------------------------- host prep

def _prep_host(x, edge_index, n_cores=N_CORES, tpc=TPC):
    N, FIN = x.shape
    npc = tpc * 128
    npad = n_cores * npc
    assert npad >= N
    src = np.asarray(edge_index[0], np.int64)
    dst = np.asarray(edge_index[1], np.int64)
    deg = np.bincount(dst, minlength=npad)

    order = np.argsort(-deg, kind="stable")
    node_of_row = np.empty(npad, np.int64)
    ntiles = npad // 128
    k_arr = np.arange(ntiles)
    c_of_tile = k_arr % n_cores
    t_of_tile = k_arr // n_cores
    for k in range(ntiles):
        c, t = int(c_of_tile[k]), int(t_of_tile[k])
        node_of_row[c * npc + t * 128:(c * npc + t * 128) + 128] = \
            order[k * 128:(k + 1) * 128]
    row_of_node = np.empty(npad, np.int64)
    row_of_node[node_of_row] = np.arange(npad)

    # shared per-tile round counts: max degree within rank window of 8 tiles
    K = np.maximum(1, deg[order[np.arange(tpc) * (n_cores * 128)]]).astype(
        np.int64)
    Kcum = np.concatenate([[0], np.cumsum(K)]).astype(np.int64)
    KTOT = int(K[-1] + Kcum[-2]) if tpc > 1 else int(K[0])
    KTOT = int(Kcum[-1])

    # per-edge slot assignment (vectorized)
    erow = row_of_node[dst]
    eord = np.argsort(erow, kind="stable")
    src_s = src[eord]
    erow_s = erow[eord]
    starts = np.searchsorted(erow_s, np.arange(npad + 1))
    kidx = np.arange(len(src_s)) - np.repeat(starts[:-1],
                                             np.diff(starts))
    ec = erow_s // npc
    et = (erow_s % npc) // 128
    ep = erow_s % 128
    offs = np.full((n_cores, 128, KTOT), npad, np.int32)
    offs[ec, ep, Kcum[et] + kidx] = row_of_node[src_s].astype(np.int32)

    xT = np.zeros((n_cores, 128, npc), dtype=np.float16)
    xpad = np.zeros((npad, FIN), np.float32)
    xpad[:N] = np.asarray(x, np.float32)
    for c in range(n_cores):
        xT[c] = xpad[node_of_row[c * npc:(c + 1) * npc]].T.astype(
            np.float16)

    # chunk-major table2 row permutation (for overlapped chunked AllGather-2)
    chunk_t = 14 if tpc % 14 == 0 else (2 if tpc % 2 == 0 else tpc)
    rows_all = np.arange(npad, dtype=np.int64)
    rc = rows_all // npc
    rt = (rows_all % npc) // 128
    rrem = rows_all % 128
    rj = rt // chunk_t
    perm2 = np.empty(npad + 1, np.int32)
    perm2[rows_all] = (rj * (n_cores * chunk_t * 128) + rc * (chunk_t * 128)
                       + (rt % chunk_t) * 128 + rrem).astype(np.int32)
    perm2[npad] = npad
    offs2 = perm2[offs]

    meta = dict(K=K.tolist(), Kcum=Kcum.tolist(), KTOT=KTOT,
                node_of_row=node_of_row, N=N, n_cores=n_cores, tpc=tpc,
                npc=npc, npad=npad, chunk_t=chunk_t)
    return offs, offs2, xT, meta


def _prep_weights(W1, a_s1, a_d1, b1, W2, a_s2, a_d2, b2):
    W1 = np.asarray(W1, np.float32)
    H, C = np.asarray(a_s1).shape
    FIN = W1.shape[0]
    Ws1 = np.zeros((FIN, H), np.float32)
    Wd1 = np.zeros((FIN, H), np.float32)
    for h in range(H):
        Ws1[:, h] = W1[:, h * C:(h + 1) * C] @ np.asarray(a_s1, np.float32)[h]
        Wd1[:, h] = W1[:, h * C:(h + 1) * C] @ np.asarray(a_d1, np.float32)[h]
    W1cat = np.concatenate([W1, Ws1, Wd1], axis=1)
    W2 = np.asarray(W2, np.float32)
    HID = W1.shape[1]
    sent = np.zeros((1, ROW), np.float32)
    sent[0, HID:HID + 8] = -100.0
    return dict(
        W1cat=W1cat.astype(np.float16),
        identb=np.eye(128, dtype=np.float32).astype(np.float16),
        W2s=W2,
        b1row=np.asarray(b1, np.float32).reshape(1, HID),
        b2row=np.asarray(b2, np.float32).reshape(1, NCLASS),
        ws2row=(W2 @ np.asarray(a_s2, np.float32)[0]).reshape(1, HID),
        wd2row=(W2 @ np.asarray(a_d2, np.float32)[0]).reshape(1, HID),
        ones_row=np.ones((1, 128), np.float32),
        sent=sent.astype(np.float16),
    )


# ----------------------------------------------------------------- builder

def _build(meta, timeline=False, phases=4, tlim=None, strip=127):
    from concourse import bass, bacc, mybir, tile
    F32, BF16, I32 = mybir.dt.float32, mybir.dt.float16, mybir.dt.int32
    MULT, ADD, MAXOP, SUB = (mybir.AluOpType.mult, mybir.AluOpType.add,
                             mybir.AluOpType.max, mybir.AluOpType.subtract)
    EXPF = mybir.ActivationFunctionType.Exp
    K, Kcum, KTOT = meta["K"], meta["Kcum"], meta["KTOT"]
    tpc, npc, npad = meta["tpc"], meta["npc"], meta["npad"]
    tlim = tpc if tlim is None else tlim
    n_cores = 1 if timeline else meta["n_cores"]
    ncol = meta["n_cores"]

    nc = bacc.Bacc("TRN2", target_bir_lowering=False, debug=False,
                   num_devices=n_cores)
    xT = nc.dram_tensor("xT", [128, npc], BF16, kind="ExternalInput")
    offs = nc.dram_tensor("offs", [128, KTOT], I32, kind="ExternalInput")
    offs2 = nc.dram_tensor("offs2", [128, KTOT], I32, kind="ExternalInput")
    chunk_t = meta.get("chunk_t", tpc)
    W1cat = nc.dram_tensor("W1cat", [128, 144], BF16, kind="ExternalInput")
    identb = nc.dram_tensor("identb", [128, 128], BF16, kind="ExternalInput")
    W2s = nc.dram_tensor("W2s", [128, NCLASS], F32, kind="ExternalInput")
    sentd = nc.dram_tensor("sent", [1, ROW], BF16, kind="ExternalInput")
    H = {}
    for nm in ("b1row", "ws2row", "wd2row", "ones_row"):
        H[nm] = nc.dram_tensor(nm, [1, 128], F32, kind="ExternalInput")
    H["b2row"] = nc.dram_tensor("b2row", [1, NCLASS], F32,
                                kind="ExternalInput")
    hloc = nc.dram_tensor("hloc", [npc, ROW], BF16)
    adt1 = nc.dram_tensor("adt1", [npc, 8], F32)
    table1 = nc.dram_tensor("table1", [npad + 1, ROW], BF16)
    gloc = nc.dram_tensor("gloc", [npc, ROW], BF16)
    adt2 = nc.dram_tensor("adt2", [npc, 1], F32)
    table2 = nc.dram_tensor("table2", [npad + 1, ROW], BF16)
    out2 = nc.dram_tensor("out2", [npc, NCLASS], F32, kind="ExternalOutput")
    groups = [list(range(ncol))]

    def allgather(tc, src_t, dst_t):
        if timeline:
            for j in range(ncol):
                nc.sync.dma_start(out=dst_t[j * npc:(j + 1) * npc, :],
                                  in_=src_t[0:npc, :])
        else:
            nc.gpsimd.collective_compute(
                "AllGather", mybir.AluOpType.bypass, groups,
                ins=[src_t[0:npc, :]], outs=[dst_t[0:npad, :]])

    with tile.TileContext(nc) as tc:
        with (
            tc.tile_pool(name="const", bufs=1) as constp,
            tc.tile_pool(name="io", bufs=4) as io,
            tc.tile_pool(name="gp", bufs=3) as gp,
            tc.tile_pool(name="est", bufs=4) as est,
            tc.tile_pool(name="stg", bufs=4) as stg,
        ):
            # ---- constants
            ident = constp.tile([128, 128], BF16, name="ident")
            nc.sync.dma_start(out=ident[:], in_=identb[:, :])
            w1c = constp.tile([128, 144], BF16, name="w1c")
            nc.sync.dma_start(out=w1c[:], in_=W1cat[:, :])
            w2t = constp.tile([128, NCLASS], F32, name="w2t")
            nc.sync.dma_start(out=w2t[:], in_=W2s[:, :])
            onesr = constp.tile([1, 128], F32, name="onesr")
            nc.sync.dma_start(out=onesr[:], in_=H["ones_row"].ap())
            sent_t = constp.tile([1, ROW], BF16, name="sent_t")
            nc.sync.dma_start(out=sent_t[:], in_=sentd[:, :])
            rows = constp.tile([1, 384 + NCLASS], F32, name="rows")
            for i, nm in enumerate(["b1row", "ws2row", "wd2row"]):
                nc.sync.dma_start(out=rows[:, i * 128:(i + 1) * 128],
                                  in_=H[nm].ap())
            nc.sync.dma_start(out=rows[:, 384:384 + NCLASS],
                              in_=H["b2row"].ap())
            C = {}
            with tc.tile_pool(name="psC", bufs=2, space="PSUM") as psC:
                for nm, lo, n in (("b1F", 0, 128), ("ws2F", 128, 128),
                                  ("wd2F", 256, 128), ("b2F", 384, NCLASS)):
                    t_ = constp.tile([128, n], F32, name=nm)
                    prt = psC.tile([128, 128], F32, tag="rep", name="prt")
                    nc.tensor.matmul(prt[:, 0:n], lhsT=onesr[:],
                                     rhs=rows[:, lo:lo + n],
                                     start=True, stop=True)
                    nc.vector.tensor_copy(t_[:], prt[:, 0:n])
                    C[nm] = t_

            # ---- dense phase (own tiles only)
            with (
                tc.tile_pool(name="xin", bufs=1) as xin,
                tc.tile_pool(name="psD", bufs=4, space="PSUM") as psD,
            ):
                xfull = xin.tile([128, npc], BF16, name="xfull")
                nc.sync.dma_start(out=xfull[:], in_=xT[:, :])
                DB = 7  # tiles per batched write (98 = 14 x 7)
                for t0 in range(0, tpc, DB):
                    nb = min(DB, tpc - t0)
                    hst = stg.tile([128, DB * ROW], BF16, tag="hst",
                                   name="hst")
                    ast = stg.tile([128, DB * 8], F32, tag="ast", name="ast")
                    for j in range(nb):
                        t = t0 + j
                        ph = psD.tile([128, 144], F32, tag="ph", name="ph")
                        nc.tensor.matmul(ph[:], lhsT=xfull[:, t*128:(t+1)*128],
                                         rhs=w1c[:], start=True, stop=True)
                        if t % 2 == 0:
                            nc.vector.tensor_copy(hst[:, j*ROW:(j+1)*ROW],
                                                  ph[:, 0:ROW])
                        else:
                            nc.scalar.copy(hst[:, j*ROW:(j+1)*ROW],
                                           ph[:, 0:ROW])
                        nc.vector.tensor_copy(ast[:, j*8:(j+1)*8],
                                              ph[:, 136:144])
                    nc.sync.dma_start(
                        out=hloc[t0*128:(t0+nb)*128, :].rearrange(
                            "(t p) f -> p t f", p=128),
                        in_=hst[:, 0:nb*ROW].rearrange("p (t f) -> p t f",
                                                       f=ROW))
                    nc.sync.dma_start(
                        out=adt1[t0*128:(t0+nb)*128, :].rearrange(
                            "(t p) f -> p t f", p=128),
                        in_=ast[:, 0:nb*8].rearrange("p (t f) -> p t f", f=8))

            allgather(tc, hloc, table1)
            nc.sync.dma_start(out=table1[npad:npad + 1, :], in_=sent_t[:])

            # ---- layer-1 edge phase
            with tc.tile_pool(name="psA", bufs=2, space="PSUM") as psA:
                for t in range(tlim if phases >= 2 else 0):
                    Kt = K[t]
                    c0 = Kcum[t]
                    ot = io.tile([128, Kt], I32, tag="ot", name="ot")
                    nc.sync.dma_start(out=ot[:], in_=offs[:, c0:c0 + Kt])
                    adt = io.tile([128, 8], F32, tag="adt", name="adt")
                    nc.sync.dma_start(out=adt[:], in_=adt1[t*128:(t+1)*128, :])
                    hD = io.tile([128, ROW], BF16, tag="hD", name="hD")
                    nc.sync.dma_start(out=hD[:], in_=hloc[t*128:(t+1)*128, :])
                    G = gp.tile([128, Kt * ROW], BF16, tag="G", name="G")
                    if strip & 1:
                        for k in range(Kt):
                            nc.gpsimd.indirect_dma_start(
                                out=G[:, k*ROW:(k+1)*ROW], out_offset=None,
                                in_=table1[:, :],
                                in_offset=bass.IndirectOffsetOnAxis(
                                    ap=ot[:, k:k + 1], axis=0))
                    if strip != 127:
                        gc0 = stg.tile([128, ROW], BF16, tag="gcat", name="gc0")
                        if strip & 1:
                            nc.vector.tensor_copy(gc0[:], G[:, 0:ROW])
                        else:
                            nc.vector.tensor_copy(gc0[:], hD[:])
                        nc.sync.dma_start(out=gloc[t*128:(t+1)*128, :],
                                          in_=gc0[:])
                    # e = a_src + a_dst ; leaky ; exp  (views ordered [p,h,k])
                    Gas = G[:].rearrange("p (k f) -> p f k", k=Kt)[:, 128:136, :]
                    E = est.tile([128, Kt * 8], F32, tag="E", name="E")
                    Ev = E[:].rearrange("p (h k) -> p h k", h=8)
                    if strip & 2:
                        nc.vector.tensor_tensor(
                            Ev, Gas, adt[:].to_broadcast([128, 8, Kt]), ADD)
                        E2 = est.tile([128, Kt * 8], F32, tag="E2", name="E2")
                        nc.vector.tensor_scalar_mul(E2[:], E[:], NEG)
                        nc.vector.tensor_tensor(E[:], E[:], E2[:], MAXOP)
                        eS = est.tile([128, 8], F32, tag="eS", name="eS")
                        nc.vector.tensor_tensor(eS[:], hD[:, 128:136], adt[:],
                                                ADD)
                        eS2 = est.tile([128, 8], F32, tag="eS2", name="eS2")
                        nc.vector.tensor_scalar_mul(eS2[:], eS[:], NEG)
                        nc.vector.tensor_tensor(eS[:], eS[:], eS2[:], MAXOP)
                    if strip & 4:
                        nc.scalar.activation(Gas, Ev, EXPF)
                        nc.scalar.activation(hD[:, 128:136], eS[:], EXPF)
                    # msg = h * ex
                    if strip & 8:
                        Gh = G[:].rearrange("p (k f) -> p k f", k=Kt)[:, :, 0:128] \
                            .rearrange("p k (h c) -> p k h c", c=16)
                        Gex = G[:].rearrange("p (k f) -> p k f",
                                             k=Kt)[:, :, 128:136]
                        nc.vector.tensor_tensor(
                            Gh, Gh, Gex.to_broadcast([128, Kt, 8, 16]), MULT)
                        hDh = hD[:, 0:128].rearrange("p (h c) -> p h c", c=16)
                        nc.vector.tensor_tensor(
                            hDh, hDh,
                            hD[:, 128:136].to_broadcast([128, 8, 16]), MULT)
                    if not (strip & 16):
                        continue
                    # aggregate: K+1 identity matmuls
                    ACC = psA.tile([128, ROW], F32, tag="ACC", name="ACC")
                    for k in range(Kt):
                        nc.tensor.matmul(ACC[:], lhsT=ident[:],
                                         rhs=G[:, k*ROW:(k+1)*ROW],
                                         start=(k == 0), stop=False)
                    nc.tensor.matmul(ACC[:], lhsT=ident[:], rhs=hD[:],
                                     start=False, stop=True)
                    if not (strip & 32):
                        gc1 = stg.tile([128, ROW], F32, tag="g1", name="gc1")
                        nc.vector.tensor_copy(gc1[:], ACC[:])
                        nc.sync.dma_start(out=gloc[t*128:(t+1)*128, :],
                                          in_=gc1[:].bitcast(BF16)[:, 0:ROW])
                        continue
                    # close: normalize, +b1, ELU, a2 scalars
                    rcp = est.tile([128, 8], F32, tag="rcp", name="rcp")
                    nc.vector.reciprocal(rcp[:], ACC[:, 128:136])
                    g1 = stg.tile([128, 128], F32, tag="g1", name="g1")
                    g1v = g1[:].rearrange("p (h c) -> p h c", c=16)
                    nc.vector.tensor_tensor(
                        g1v, ACC[:, 0:128].rearrange("p (h c) -> p h c", c=16),
                        rcp[:].to_broadcast([128, 8, 16]), MULT)
                    nc.vector.tensor_tensor(g1[:], g1[:], C["b1F"][:], ADD)
                    mt = est.tile([128, 128], F32, tag="mt", name="mt")
                    nc.vector.tensor_scalar_min(mt[:], g1[:], 0.0)
                    emt = est.tile([128, 128], F32, tag="emt", name="emt")
                    nc.scalar.activation(emt[:], mt[:], EXPF)
                    nc.vector.tensor_scalar(g1[:], g1[:], 0.0, 1.0, MAXOP, SUB)
                    nc.vector.tensor_tensor(g1[:], g1[:], emt[:], ADD)
                    gcat = stg.tile([128, ROW], BF16, tag="gcat", name="gcat")
                    nc.vector.tensor_copy(gcat[:, 0:128], g1[:])
                    nc.vector.memset(gcat[:, 129:136], 0.0)
                    if strip & 64:
                        dmy = est.tile([128, 128], F32, tag="dmy", name="dmy")
                        a2s = est.tile([128, 1], F32, tag="a2s", name="a2s")
                        nc.vector.tensor_tensor(dmy[:], g1[:], C["ws2F"][:],
                                                MULT)
                        nc.vector.tensor_reduce(a2s[:], dmy[:],
                                                mybir.AxisListType.X, ADD)
                        a2d = stg.tile([128, 1], F32, tag="a2d", name="a2d")
                        nc.vector.tensor_tensor(dmy[:], g1[:], C["wd2F"][:],
                                                MULT)
                        nc.vector.tensor_reduce(a2d[:], dmy[:],
                                                mybir.AxisListType.X, ADD)
                        nc.vector.tensor_copy(gcat[:, 128:129], a2s[:])
                        nc.sync.dma_start(out=adt2[t*128:(t+1)*128, :],
                                          in_=a2d[:])
                    else:
                        nc.vector.memset(gcat[:, 128:129], 0.0)
                    nc.sync.dma_start(out=gloc[t*128:(t+1)*128, :],
                                      in_=gcat[:])

            if phases >= 3:
                CH = chunk_t * 128
                for j in range(0, tpc // chunk_t):
                    r0 = j * CH
                    o0 = j * ncol * CH
                    if timeline:
                        for cc in range(ncol):
                            nc.sync.dma_start(
                                out=table2[o0 + cc*CH:o0 + (cc+1)*CH, :],
                                in_=gloc[r0:r0 + CH, :])
                    else:
                        nc.gpsimd.collective_compute(
                            "AllGather", mybir.AluOpType.bypass, groups,
                            ins=[gloc[r0:r0 + CH, :]],
                            outs=[table2[o0:o0 + ncol * CH, :]])
                nc.sync.dma_start(out=table2[npad:npad + 1, :], in_=sent_t[:])
            if phases < 4:
                for t in range(tpc):
                    nc.sync.dma_start(out=out2[t*128:(t+1)*128, :],
                                      in_=C["b2F"][:])

            # ---- layer-2 edge phase
            with (
                tc.tile_pool(name="psT", bufs=2, space="PSUM") as psT,
                tc.tile_pool(name="psO", bufs=2, space="PSUM") as psO,
            ):
                for t in range(tlim if phases >= 4 else 0):
                    Kt = K[t]
                    c0 = Kcum[t]
                    ot2 = io.tile([128, Kt], I32, tag="ot", name="ot2")
                    nc.sync.dma_start(out=ot2[:], in_=offs2[:, c0:c0 + Kt])
                    a2t = io.tile([128, 1], F32, tag="a2t", name="a2t")
                    nc.sync.dma_start(out=a2t[:], in_=adt2[t*128:(t+1)*128, :])
                    gD = io.tile([128, ROW], BF16, tag="hD", name="gD")
                    nc.sync.dma_start(out=gD[:], in_=gloc[t*128:(t+1)*128, :])
                    G2 = gp.tile([128, Kt * ROW], BF16, tag="G", name="G2")
                    for k in range(Kt):
                        nc.gpsimd.indirect_dma_start(
                            out=G2[:, k*ROW:(k+1)*ROW], out_offset=None,
                            in_=table2[:, :],
                            in_offset=bass.IndirectOffsetOnAxis(
                                ap=ot2[:, k:k + 1], axis=0))
                    Gas2 = G2[:].rearrange("p (k f) -> p f k", k=Kt)[:, 128:129, :]
                    E2a = est.tile([128, Kt], F32, tag="E", name="E2a")
                    E2v = E2a[:].rearrange("p (h k) -> p h k", h=1)
                    nc.vector.tensor_tensor(E2v, Gas2,
                                            a2t[:].to_broadcast([128, 1, Kt]),
                                            ADD)
                    E2b = est.tile([128, Kt], F32, tag="E2", name="E2b")
                    nc.vector.tensor_scalar_mul(E2b[:], E2a[:], NEG)
                    nc.vector.tensor_tensor(E2a[:], E2a[:], E2b[:], MAXOP)
                    nc.scalar.activation(Gas2, E2v, EXPF)
                    e2S = est.tile([128, 1], F32, tag="eS", name="e2S")
                    nc.vector.tensor_tensor(e2S[:], gD[:, 128:129], a2t[:],
                                            ADD)
                    e2S2 = est.tile([128, 1], F32, tag="eS2", name="e2S2")
                    nc.vector.tensor_scalar_mul(e2S2[:], e2S[:], NEG)
                    nc.vector.tensor_tensor(e2S[:], e2S[:], e2S2[:], MAXOP)
                    nc.scalar.activation(gD[:, 128:129], e2S[:], EXPF)
                    # s2 = sum_k ex + ex_self
                    s2 = est.tile([128, 1], F32, tag="s2", name="s2")
                    Gex2 = G2[:].rearrange("p (k f) -> p k f", k=Kt)[:, :, 128]
                    nc.vector.tensor_reduce(s2[:], Gex2,
                                            mybir.AxisListType.X, ADD)
                    nc.vector.tensor_tensor(s2[:], s2[:], gD[:, 128:129], ADD)
                    rcp2 = est.tile([128, 1], F32, tag="rcp2", name="rcp2")
                    nc.vector.reciprocal(rcp2[:], s2[:])
                    # msg = g * ex
                    for k in range(Kt):
                        nc.vector.tensor_tensor(
                            G2[:, k*ROW:k*ROW + 128], G2[:, k*ROW:k*ROW + 128],
                            G2[:, k*ROW + 128:k*ROW + 129].to_broadcast(
                                [128, 128]), MULT)
                    nc.vector.tensor_tensor(
                        gD[:, 0:128], gD[:, 0:128],
                        gD[:, 128:129].to_broadcast([128, 128]), MULT)
                    PT = psT.tile([128, 128], F32, tag="PT", name="PT")
                    for k in range(Kt):
                        nc.tensor.matmul(PT[:], lhsT=G2[:, k*ROW:k*ROW + 128],
                                         rhs=ident[:], start=(k == 0),
                                         stop=False)
                    nc.tensor.matmul(PT[:], lhsT=gD[:, 0:128], rhs=ident[:],
                                     start=False, stop=True)
                    aggT = stg.tile([128, 128], F32, tag="aggT", name="aggT")
                    nc.vector.tensor_copy(aggT[:], PT[:])
                    PO = psO.tile([128, NCLASS], F32, tag="PO", name="PO")
                    nc.tensor.matmul(PO[:], lhsT=aggT[:], rhs=w2t[:],
                                     start=True, stop=True)
                    o2 = stg.tile([128, NCLASS], F32, tag="o2", name="o2")
                    nc.vector.tensor_tensor(
                        o2[:], PO[:], rcp2[:].to_broadcast([128, NCLASS]),
                        MULT)
                    nc.vector.tensor_tensor(o2[:], o2[:], C["b2F"][:], ADD)
                    nc.sync.dma_start(out=out2[t*128:(t+1)*128, :], in_=o2[:])
    nc.compile()
    return nc


# ----------------------------------------------------------------- runner

def _device_forward(inputs):
    from concourse import bass_utils
    x = np.asarray(inputs["x"], np.float32)
    ei = np.asarray(inputs["edge_index"])
    offs, offs2, xT, meta = _prep_host(x, ei)
    wd = _prep_weights(inputs["W1"], inputs["att_src1"], inputs["att_dst1"],
                       inputs["b1"], inputs["W2"], inputs["att_src2"],
                       inputs["att_dst2"], inputs["b2"])
    if "nc" not in _CACHE:
        _CACHE["nc"] = _build(meta)
    nc = _CACHE["nc"]
    in_maps = [dict(xT=xT[c], offs=offs[c], offs2=offs2[c], **wd)
               for c in range(N_CORES)]
    res = None
    for attempt in range(3):
        try:
            res = bass_utils.run_bass_kernel_spmd(
                nc, in_maps, core_ids=list(range(N_CORES)))
            break
        except Exception as e:
            sys.stderr.write(f"[kernel] exec attempt {attempt} failed "
                             f"({e!r}); retrying\n")
            import time as _time
            _time.sleep(2.0)
    if res is None:
        res = bass_utils.run_bass_kernel_spmd(
            nc, in_maps, core_ids=list(range(N_CORES)))
    npc, N = meta["npc"], meta["N"]
    out_full = np.zeros((meta["npad"], NCLASS), np.float32)
    for c in range(N_CORES):
        out_full[meta["node_of_row"][c * npc:(c + 1) * npc]] = \
            res.results[c]["out2"]
    return out_full[:N]


def kernel(**inputs):
    for attempt in range(2):
        try:
            out = _device_forward(inputs)
            if not np.all(np.isfinite(out)):
                raise RuntimeError("non-finite device output")
            return out
        except Exception as e:
            sys.stderr.write(f"[kernel] device attempt {attempt} failed "
                             f"({e!r})\n")
    sys.stderr.write("[kernel] device path failed; numpy fallback\n")
    return _np_forward(
        np.asarray(inputs["x"], np.float32), inputs["edge_index"],
        inputs["W1"], inputs["att_src1"], inputs["att_dst1"], inputs["b1"],
        inputs["W2"], inputs["att_src2"], inputs["att_dst2"], inputs["b2"])


# revision 39
# speedup vs baseline: 1.0147x; 1.0005x over previous
"""2-layer GAT (PyG-style) on 8 Trainium2 NeuronCores via Bass/Tile.

Self-contained: kernel(**inputs) -> [100000, 40] float32.

Design (v2, "slot-rounds"):
  Nodes sorted by in-degree, grouped into 784 tiles of 128; tiles dealt
  round-robin to 8 cores (98 dst-tiles each, degree-homogeneous so all cores
  share one per-tile round count K[t]). Edges of dst p in tile t occupy
  "slot rounds" k=0..deg-1 at partition p; padding rounds point at a sentinel
  table row whose attention logit is -100 (exp ~= 0 even after leaky-relu).

  One SPMD program per core:
    dense:  h|a_src|a_dst = xT_c @ [W1|Ws1|Wd1] for the core's own 98 tiles
            -> hloc [NPC,136] fp16, adt1 [NPC,8] f32
    AG1:    AllGather hloc -> table1 [NPAD,136] (+sentinel row)
    L1 edge: per tile: K indirect row-gathers (128 rows each) from table1,
            e = a_src[src]+a_dst[dst], leaky-relu, exp (no max-sub needed;
            |e| small), msg = h*ex, aggregation = K+1 identity matmuls
            accumulating [h*ex | ex] into PSUM (self-loop via the tile's own
            sequential rows), normalize, +b1, ELU -> g; a2 scalars by
            linearity (g.(W2@att2)) -> gloc [NPC,136] fp16, adt2 [NPC,1]
    AG2:    AllGather gloc -> table2 (+sentinel)
    L2 edge: same gather structure in g-space (1 head); aggregation
            transposed (lhsT=g*ex chunks, rhs=I) -> aggT; out = aggT@W2/s+b2

Falls back to a pure-numpy forward if the device path fails.
"""
import sys
sys.path.insert(0, "/opt/trn_rl_repo")
sys.path.insert(0, "/root/.axon_site")
import numpy as np

N_CORES = 8
TPC = 98
NCLASS = 40
NEG = 0.2
ROW = 136

_CACHE = {}


# ----------------------------------------------------------------- numpy ref

def _np_forward(x, edge_index, W1, a_s1, a_d1, b1, W2, a_s2, a_d2, b2):
    N = x.shape[0]
    src = np.concatenate([np.asarray(edge_index[0], np.int64), np.arange(N)])
    dst = np.concatenate([np.asarray(edge_index[1], np.int64), np.arange(N)])
    o = np.argsort(dst, kind="stable")
    src, dst = src[o], dst[o]
    starts = np.searchsorted(dst, np.arange(N))

    def gat(xx, W, a_s, a_d, bb, concat):
        H, C = a_s.shape
        h = (xx @ np.asarray(W, xx.dtype)).reshape(-1, H, C)
        asr = np.einsum("nhc,hc->nh", h, np.asarray(a_s, xx.dtype))
        ads = np.einsum("nhc,hc->nh", h, np.asarray(a_d, xx.dtype))
        e = asr[src] + ads[dst]
        e = np.where(e >= 0, e, NEG * e)
        ex = np.exp(e)
        s = np.add.reduceat(ex, starts, axis=0)
        alpha = ex / s[dst]
        msg = (h[src] * alpha[:, :, None]).reshape(len(src), -1)
        out = np.add.reduceat(msg, starts, axis=0).reshape(N, H, C)
        out = out.reshape(N, H * C) if concat else out.mean(axis=1)
        return out + np.asarray(bb, xx.dtype)

    h = gat(x.astype(np.float64), W1, a_s1, a_d1, b1, True)
    h = np.where(h > 0, h, np.exp(np.minimum(h, 0)) - 1.0)
    out = gat(h, W2, a_s2, a_d2, b2, False)
    return out.astype(np.float32)


# ----------------------------------------------------------------- host prep

def _prep_host(x, edge_index, n_cores=N_CORES, tpc=TPC):
    N, FIN = x.shape
    npc = tpc * 128
    npad = n_cores * npc
    assert npad >= N
    src = np.asarray(edge_index[0], np.int64)
    dst = np.asarray(edge_index[1], np.int64)
    deg = np.bincount(dst, minlength=npad)

    order = np.argsort(-deg, kind="stable")
    node_of_row = np.empty(npad, np.int64)
    ntiles = npad // 128
    k_arr = np.arange(ntiles)
    c_of_tile = k_arr % n_cores
    t_of_tile = k_arr // n_cores
    for k in range(ntiles):
        c, t = int(c_of_tile[k]), int(t_of_tile[k])
        node_of_row[c * npc + t * 128:(c * npc + t * 128) + 128] = \
            order[k * 128:(k + 1) * 128]
    row_of_node = np.empty(npad, np.int64)
    row_of_node[node_of_row] = np.arange(npad)

    # shared per-tile round counts: max degree within rank window of 8 tiles
    K = np.maximum(1, deg[order[np.arange(tpc) * (n_cores * 128)]]).astype(
        np.int64)
    Kcum = np.concatenate([[0], np.cumsum(K)]).astype(np.int64)
    KTOT = int(K[-1] + Kcum[-2]) if tpc > 1 else int(K[0])
    KTOT = int(Kcum[-1])

    # per-edge slot assignment (vectorized)
    erow = row_of_node[dst]
    eord = np.argsort(erow, kind="stable")
    src_s = src[eord]
    erow_s = erow[eord]
    starts = np.searchsorted(erow_s, np.arange(npad + 1))
    kidx = np.arange(len(src_s)) - np.repeat(starts[:-1],
                                             np.diff(starts))
    ec = erow_s // npc
    et = (erow_s % npc) // 128
    ep = erow_s % 128
    offs = np.full((n_cores, 128, KTOT), npad, np.int32)
    offs[ec, ep, Kcum[et] + kidx] = row_of_node[src_s].astype(np.int32)

    xT = np.zeros((n_cores, 128, npc), dtype=np.float16)
    xpad = np.zeros((npad, FIN), np.float32)
    xpad[:N] = np.asarray(x, np.float32)
    for c in range(n_cores):
        xT[c] = xpad[node_of_row[c * npc:(c + 1) * npc]].T.astype(
            np.float16)

    # chunk-major table2 row permutation (for overlapped chunked AllGather-2)
    chunk_t = 14 if tpc % 14 == 0 else (2 if tpc % 2 == 0 else tpc)
    rows_all = np.arange(npad, dtype=np.int64)
    rc = rows_all // npc
    rt = (rows_all % npc) // 128
    rrem = rows_all % 128
    rj = rt // chunk_t
    perm2 = np.empty(npad + 1, np.int32)
    perm2[rows_all] = (rj * (n_cores * chunk_t * 128) + rc * (chunk_t * 128)
                       + (rt % chunk_t) * 128 + rrem).astype(np.int32)
    perm2[npad] = npad
    offs2 = perm2[offs]

    meta = dict(K=K.tolist(), Kcum=Kcum.tolist(), KTOT=KTOT,
                node_of_row=node_of_row, N=N, n_cores=n_cores, tpc=tpc,
                npc=npc, npad=npad, chunk_t=chunk_t)
    return offs, offs2, xT, meta


def _prep_weights(W1, a_s1, a_d1, b1, W2, a_s2, a_d2, b2):
    W1 = np.asarray(W1, np.float32)
    H, C = np.asarray(a_s1).shape
    FIN = W1.shape[0]
    Ws1 = np.zeros((FIN, H), np.float32)
    Wd1 = np.zeros((FIN, H), np.float32)
    for h in range(H):
        Ws1[:, h] = W1[:, h * C:(h + 1) * C] @ np.asarray(a_s1, np.float32)[h]
        Wd1[:, h] = W1[:, h * C:(h + 1) * C] @ np.asarray(a_d1, np.float32)[h]
    W1cat = np.concatenate([W1, Ws1, Wd1], axis=1)
    W2 = np.asarray(W2, np.float32)
    HID = W1.shape[1]
    sent = np.zeros((1, ROW), np.float32)
    sent[0, HID:HID + 8] = -100.0
    return dict(
        W1cat=W1cat.astype(np.float16),
        identb=np.eye(128, dtype=np.float32).astype(np.float16),
        W2s=W2,
        b1row=np.asarray(b1, np.float32).reshape(1, HID),
        b2row=np.asarray(b2, np.float32).reshape(1, NCLASS),
        ws2row=(W2 @ np.asarray(a_s2, np.float32)[0]).reshape(1, HID),
        wd2row=(W2 @ np.asarray(a_d2, np.float32)[0]).reshape(1, HID),
        ones_row=np.ones((1, 128), np.float32),
        sent=sent.astype(np.float16),
    )


# ----------------------------------------------------------------- builder

def _build(meta, timeline=False, phases=4, tlim=None, strip=127):
    from concourse import bass, bacc, mybir, tile
    F32, BF16, I32 = mybir.dt.float32, mybir.dt.float16, mybir.dt.int32
    MULT, ADD, MAXOP, SUB = (mybir.AluOpType.mult, mybir.AluOpType.add,
                             mybir.AluOpType.max, mybir.AluOpType.subtract)
    EXPF = mybir.ActivationFunctionType.Exp
    K, Kcum, KTOT = meta["K"], meta["Kcum"], meta["KTOT"]
    tpc, npc, npad = meta["tpc"], meta["npc"], meta["npad"]
    tlim = tpc if tlim is None else tlim
    n_cores = 1 if timeline else meta["n_cores"]
    ncol = meta["n_cores"]

    nc = bacc.Bacc("TRN2", target_bir_lowering=False, debug=False,
                   num_devices=n_cores)
    xT = nc.dram_tensor("xT", [128, npc], BF16, kind="ExternalInput")
    offs = nc.dram_tensor("offs", [128, KTOT], I32, kind="ExternalInput")
    offs2 = nc.dram_tensor("offs2", [128, KTOT], I32, kind="ExternalInput")
    chunk_t = meta.get("chunk_t", tpc)
    W1cat = nc.dram_tensor("W1cat", [128, 144], BF16, kind="ExternalInput")
    identb = nc.dram_tensor("identb", [128, 128], BF16, kind="ExternalInput")
    W2s = nc.dram_tensor("W2s", [128, NCLASS], F32, kind="ExternalInput")
    sentd = nc.dram_tensor("sent", [1, ROW], BF16, kind="ExternalInput")
    H = {}
    for nm in ("b1row", "ws2row", "wd2row", "ones_row"):
        H[nm] = nc.dram_tensor(nm, [1, 128], F32, kind="ExternalInput")
    H["b2row"] = nc.dram_tensor("b2row", [1, NCLASS], F32,
                                kind="ExternalInput")
    hloc = nc.dram_tensor("hloc", [npc, ROW], BF16)
    adt1 = nc.dram_tensor("adt1", [npc, 8], F32)
    table1 = nc.dram_tensor("table1", [npad + 1, ROW], BF16)
    gloc = nc.dram_tensor("gloc", [npc, ROW], BF16)
    adt2 = nc.dram_tensor("adt2", [npc, 1], F32)
    table2 = nc.dram_tensor("table2", [npad + 1, ROW], BF16)
    out2 = nc.dram_tensor("out2", [npc, NCLASS], F32, kind="ExternalOutput")
    groups = [list(range(ncol))]

    def allgather(tc, src_t, dst_t):
        if timeline:
            for j in range(ncol):
                nc.sync.dma_start(out=dst_t[j * npc:(j + 1) * npc, :],
                                  in_=src_t[0:npc, :])
        else:
            nc.gpsimd.collective_compute(
                "AllGather", mybir.AluOpType.bypass, groups,
                ins=[src_t[0:npc, :]], outs=[dst_t[0:npad, :]])

    with tile.TileContext(nc) as tc:
        with (
            tc.tile_pool(name="const", bufs=1) as constp,
            tc.tile_pool(name="io", bufs=4) as io,
            tc.tile_pool(name="gp", bufs=3) as gp,
            tc.tile_pool(name="est", bufs=4) as est,
            tc.tile_pool(name="stg", bufs=4) as stg,
        ):
            # ---- constants
            ident = constp.tile([128, 128], BF16, name="ident")
            nc.sync.dma_start(out=ident[:], in_=identb[:, :])
            w1c = constp.tile([128, 144], BF16, name="w1c")
            nc.sync.dma_start(out=w1c[:], in_=W1cat[:, :])
            w2t = constp.tile([128, NCLASS], F32, name="w2t")
            nc.sync.dma_start(out=w2t[:], in_=W2s[:, :])
            onesr = constp.tile([1, 128], F32, name="onesr")
            nc.sync.dma_start(out=onesr[:], in_=H["ones_row"].ap())
            sent_t = constp.tile([1, ROW], BF16, name="sent_t")
            nc.sync.dma_start(out=sent_t[:], in_=sentd[:, :])
            rows = constp.tile([1, 384 + NCLASS], F32, name="rows")
            for i, nm in enumerate(["b1row", "ws2row", "wd2row"]):
                nc.sync.dma_start(out=rows[:, i * 128:(i + 1) * 128],
                                  in_=H[nm].ap())
            nc.sync.dma_start(out=rows[:, 384:384 + NCLASS],
                              in_=H["b2row"].ap())
            C = {}
            with tc.tile_pool(name="psC", bufs=2, space="PSUM") as psC:
                for nm, lo, n in (("b1F", 0, 128), ("ws2F", 128, 128),
                                  ("wd2F", 256, 128), ("b2F", 384, NCLASS)):
                    t_ = constp.tile([128, n], F32, name=nm)
                    prt = psC.tile([128, 128], F32, tag="rep", name="prt")
                    nc.tensor.matmul(prt[:, 0:n], lhsT=onesr[:],
                                     rhs=rows[:, lo:lo + n],
                                     start=True, stop=True)
                    nc.vector.tensor_copy(t_[:], prt[:, 0:n])
                    C[nm] = t_

            # ---- dense phase (own tiles only)
            with (
                tc.tile_pool(name="xin", bufs=1) as xin,
                tc.tile_pool(name="psD", bufs=4, space="PSUM") as psD,
            ):
                xfull = xin.tile([128, npc], BF16, name="xfull")
                nc.sync.dma_start(out=xfull[:], in_=xT[:, :])
                DB = 7  # tiles per batched write (98 = 14 x 7)
                for t0 in range(0, tpc, DB):
                    nb = min(DB, tpc - t0)
                    hst = stg.tile([128, DB * ROW], BF16, tag="hst",
                                   name="hst")
                    ast = stg.tile([128, DB * 8], F32, tag="ast", name="ast")
                    for j in range(nb):
                        t = t0 + j
                        ph = psD.tile([128, 144], F32, tag="ph", name="ph")
                        nc.tensor.matmul(ph[:], lhsT=xfull[:, t*128:(t+1)*128],
                                         rhs=w1c[:], start=True, stop=True)
                        if t % 2 == 0:
                            nc.vector.tensor_copy(hst[:, j*ROW:(j+1)*ROW],
                                                  ph[:, 0:ROW])
                        else:
                            nc.scalar.copy(hst[:, j*ROW:(j+1)*ROW],
                                           ph[:, 0:ROW])
                        nc.vector.tensor_copy(ast[:, j*8:(j+1)*8],
                                              ph[:, 136:144])
                    nc.sync.dma_start(
                        out=hloc[t0*128:(t0+nb)*128, :].rearrange(
                            "(t p) f -> p t f", p=128),
                        in_=hst[:, 0:nb*ROW].rearrange("p (t f) -> p t f",
                                                       f=ROW))
                    nc.sync.dma_start(
                        out=adt1[t0*128:(t0+nb)*128, :].rearrange(
                            "(t p) f -> p t f", p=128),
                        in_=ast[:, 0:nb*8].rearrange("p (t f) -> p t f", f=8))

            CH = chunk_t * 128
            for j in range(0, tpc // chunk_t):
                r0 = j * CH
                o0 = j * ncol * CH
                if timeline:
                    for cc in range(ncol):
                        nc.sync.dma_start(
                            out=table1[o0 + cc*CH:o0 + (cc+1)*CH, :],
                            in_=hloc[r0:r0 + CH, :])
                else:
                    nc.gpsimd.collective_compute(
                        "AllGather", mybir.AluOpType.bypass, groups,
                        ins=[hloc[r0:r0 + CH, :]],
                        outs=[table1[o0:o0 + ncol * CH, :]])
            nc.sync.dma_start(out=table1[npad:npad + 1, :], in_=sent_t[:])

            # ---- layer-1 edge phase
            with tc.tile_pool(name="psA", bufs=2, space="PSUM") as psA:
                for t in range(tlim if phases >= 2 else 0):
                    Kt = K[t]
                    c0 = Kcum[t]
                    ot = io.tile([128, Kt], I32, tag="ot", name="ot")
                    nc.sync.dma_start(out=ot[:], in_=offs2[:, c0:c0 + Kt])
                    adt = io.tile([128, 8], F32, tag="adt", name="adt")
                    nc.sync.dma_start(out=adt[:], in_=adt1[t*128:(t+1)*128, :])
                    hD = io.tile([128, ROW], BF16, tag="hD", name="hD")
                    nc.sync.dma_start(out=hD[:], in_=hloc[t*128:(t+1)*128, :])
                    G = gp.tile([128, Kt * ROW], BF16, tag="G", name="G")
                    if strip & 1:
                        for k in range(Kt):
                            nc.gpsimd.indirect_dma_start(
                                out=G[:, k*ROW:(k+1)*ROW], out_offset=None,
                                in_=table1[:, :],
                                in_offset=bass.IndirectOffsetOnAxis(
                                    ap=ot[:, k:k + 1], axis=0))
                    if strip != 127:
                        gc0 = stg.tile([128, ROW], BF16, tag="gcat", name="gc0")
                        if strip & 1:
                            nc.vector.tensor_copy(gc0[:], G[:, 0:ROW])
                        else:
                            nc.vector.tensor_copy(gc0[:], hD[:])
                        nc.sync.dma_start(out=gloc[t*128:(t+1)*128, :],
                                          in_=gc0[:])
                    # e = a_src + a_dst ; leaky ; exp  (views ordered [p,h,k])
                    Gas = G[:].rearrange("p (k f) -> p f k", k=Kt)[:, 128:136, :]
                    E = est.tile([128, Kt * 8], F32, tag="E", name="E")
                    Ev = E[:].rearrange("p (h k) -> p h k", h=8)
                    if strip & 2:
                        nc.vector.tensor_tensor(
                            Ev, Gas, adt[:].to_broadcast([128, 8, Kt]), ADD)
                        E2 = est.tile([128, Kt * 8], F32, tag="E2", name="E2")
                        nc.vector.tensor_scalar_mul(E2[:], E[:], NEG)
                        nc.vector.tensor_tensor(E[:], E[:], E2[:], MAXOP)
                        eS = est.tile([128, 8], F32, tag="eS", name="eS")
                        nc.vector.tensor_tensor(eS[:], hD[:, 128:136], adt[:],
                                                ADD)
                        eS2 = est.tile([128, 8], F32, tag="eS2", name="eS2")
                        nc.vector.tensor_scalar_mul(eS2[:], eS[:], NEG)
                        nc.vector.tensor_tensor(eS[:], eS[:], eS2[:], MAXOP)
                    if strip & 4:
                        nc.scalar.activation(Gas, Ev, EXPF)
                        nc.scalar.activation(hD[:, 128:136], eS[:], EXPF)
                    # msg = h * ex
                    if strip & 8:
                        Gh = G[:].rearrange("p (k f) -> p k f", k=Kt)[:, :, 0:128] \
                            .rearrange("p k (h c) -> p k h c", c=16)
                        Gex = G[:].rearrange("p (k f) -> p k f",
                                             k=Kt)[:, :, 128:136]
                        nc.vector.tensor_tensor(
                            Gh, Gh, Gex.to_broadcast([128, Kt, 8, 16]), MULT)
                        hDh = hD[:, 0:128].rearrange("p (h c) -> p h c", c=16)
                        nc.vector.tensor_tensor(
                            hDh, hDh,
                            hD[:, 128:136].to_broadcast([128, 8, 16]), MULT)
                    if not (strip & 16):
                        continue
                    # aggregate: K+1 identity matmuls
                    ACC = psA.tile([128, ROW], F32, tag="ACC", name="ACC")
                    for k in range(Kt):
                        nc.tensor.matmul(ACC[:], lhsT=ident[:],
                                         rhs=G[:, k*ROW:(k+1)*ROW],
                                         start=(k == 0), stop=False)
                    nc.tensor.matmul(ACC[:], lhsT=ident[:], rhs=hD[:],
                                     start=False, stop=True)
                    if not (strip & 32):
                        gc1 = stg.tile([128, ROW], F32, tag="g1", name="gc1")
                        nc.vector.tensor_copy(gc1[:], ACC[:])
                        nc.sync.dma_start(out=gloc[t*128:(t+1)*128, :],
                                          in_=gc1[:].bitcast(BF16)[:, 0:ROW])
                        continue
                    # close: normalize, +b1, ELU, a2 scalars
                    rcp = est.tile([128, 8], F32, tag="rcp", name="rcp")
                    nc.vector.reciprocal(rcp[:], ACC[:, 128:136])
                    g1 = stg.tile([128, 128], F32, tag="g1", name="g1")
                    g1v = g1[:].rearrange("p (h c) -> p h c", c=16)
                    nc.vector.tensor_tensor(
                        g1v, ACC[:, 0:128].rearrange("p (h c) -> p h c", c=16),
                        rcp[:].to_broadcast([128, 8, 16]), MULT)
                    nc.vector.tensor_tensor(g1[:], g1[:], C["b1F"][:], ADD)
                    mt = est.tile([128, 128], F32, tag="mt", name="mt")
                    nc.vector.tensor_scalar_min(mt[:], g1[:], 0.0)
                    emt = est.tile([128, 128], F32, tag="emt", name="emt")
                    nc.scalar.activation(emt[:], mt[:], EXPF)
                    nc.vector.tensor_scalar(g1[:], g1[:], 0.0, 1.0, MAXOP, SUB)
                    nc.vector.tensor_tensor(g1[:], g1[:], emt[:], ADD)
                    gcat = stg.tile([128, ROW], BF16, tag="gcat", name="gcat")
                    nc.vector.tensor_copy(gcat[:, 0:128], g1[:])
                    nc.vector.memset(gcat[:, 129:136], 0.0)
                    if strip & 64:
                        dmy = est.tile([128, 128], F32, tag="dmy", name="dmy")
                        a2s = est.tile([128, 1], F32, tag="a2s", name="a2s")
                        nc.vector.tensor_tensor(dmy[:], g1[:], C["ws2F"][:],
                                                MULT)
                        nc.vector.tensor_reduce(a2s[:], dmy[:],
                                                mybir.AxisListType.X, ADD)
                        a2d = stg.tile([128, 1], F32, tag="a2d", name="a2d")
                        nc.vector.tensor_tensor(dmy[:], g1[:], C["wd2F"][:],
                                                MULT)
                        nc.vector.tensor_reduce(a2d[:], dmy[:],
                                                mybir.AxisListType.X, ADD)
                        nc.vector.tensor_copy(gcat[:, 128:129], a2s[:])
                        nc.sync.dma_start(out=adt2[t*128:(t+1)*128, :],
                                          in_=a2d[:])
                    else:
                        nc.vector.memset(gcat[:, 128:129], 0.0)
                    nc.sync.dma_start(out=gloc[t*128:(t+1)*128, :],
                                      in_=gcat[:])

            if phases >= 3:
                CH = chunk_t * 128
                for j in range(0, tpc // chunk_t):
                    r0 = j * CH
                    o0 = j * ncol * CH
                    if timeline:
                        for cc in range(ncol):
                            nc.sync.dma_start(
                                out=table2[o0 + cc*CH:o0 + (cc+1)*CH, :],
                                in_=gloc[r0:r0 + CH, :])
                    else:
                        nc.gpsimd.collective_compute(
                            "AllGather", mybir.AluOpType.bypass, groups,
                            ins=[gloc[r0:r0 + CH, :]],
                            outs=[table2[o0:o0 + ncol * CH, :]])
                nc.sync.dma_start(out=table2[npad:npad + 1, :], in_=sent_t[:])
            if phases < 4:
                for t in range(tpc):
                    nc.sync.dma_start(out=out2[t*128:(t+1)*128, :],
                                      in_=C["b2F"][:])

            # ---- layer-2 edge phase
            with (
                tc.tile_pool(name="psT", bufs=2, space="PSUM") as psT,
                tc.tile_pool(name="psO", bufs=2, space="PSUM") as psO,
            ):
                for t in range(tlim if phases >= 4 else 0):
                    Kt = K[t]
                    c0 = Kcum[t]
                    ot2 = io.tile([128, Kt], I32, tag="ot", name="ot2")
                    nc.sync.dma_start(out=ot2[:], in_=offs2[:, c0:c0 + Kt])
                    a2t = io.tile([128, 1], F32, tag="a2t", name="a2t")
                    nc.sync.dma_start(out=a2t[:], in_=adt2[t*128:(t+1)*128, :])
                    gD = io.tile([128, ROW], BF16, tag="hD", name="gD")
                    nc.sync.dma_start(out=gD[:], in_=gloc[t*128:(t+1)*128, :])
                    G2 = gp.tile([128, Kt * ROW], BF16, tag="G", name="G2")
                    for k in range(Kt):
                        nc.gpsimd.indirect_dma_start(
                            out=G2[:, k*ROW:(k+1)*ROW], out_offset=None,
                            in_=table2[:, :],
                            in_offset=bass.IndirectOffsetOnAxis(
                                ap=ot2[:, k:k + 1], axis=0))
                    Gas2 = G2[:].rearrange("p (k f) -> p f k", k=Kt)[:, 128:129, :]
                    E2a = est.tile([128, Kt], F32, tag="E", name="E2a")
                    E2v = E2a[:].rearrange("p (h k) -> p h k", h=1)
                    nc.vector.tensor_tensor(E2v, Gas2,
                                            a2t[:].to_broadcast([128, 1, Kt]),
                                            ADD)
                    E2b = est.tile([128, Kt], F32, tag="E2", name="E2b")
                    nc.vector.tensor_scalar_mul(E2b[:], E2a[:], NEG)
                    nc.vector.tensor_tensor(E2a[:], E2a[:], E2b[:], MAXOP)
                    nc.scalar.activation(Gas2, E2v, EXPF)
                    e2S = est.tile([128, 1], F32, tag="eS", name="e2S")
                    nc.vector.tensor_tensor(e2S[:], gD[:, 128:129], a2t[:],
                                            ADD)
                    e2S2 = est.tile([128, 1], F32, tag="eS2", name="e2S2")
                    nc.vector.tensor_scalar_mul(e2S2[:], e2S[:], NEG)
                    nc.vector.tensor_tensor(e2S[:], e2S[:], e2S2[:], MAXOP)
                    nc.scalar.activation(gD[:, 128:129], e2S[:], EXPF)
                    # s2 = sum_k ex + ex_self
                    s2 = est.tile([128, 1], F32, tag="s2", name="s2")
                    Gex2 = G2[:].rearrange("p (k f) -> p k f", k=Kt)[:, :, 128]
                    nc.vector.tensor_reduce(s2[:], Gex2,
                                            mybir.AxisListType.X, ADD)
                    nc.vector.tensor_tensor(s2[:], s2[:], gD[:, 128:129], ADD)
                    rcp2 = est.tile([128, 1], F32, tag="rcp2", name="rcp2")
                    nc.vector.reciprocal(rcp2[:], s2[:])
                    # msg = g * ex
                    for k in range(Kt):
                        nc.vector.tensor_tensor(
                            G2[:, k*ROW:k*ROW + 128], G2[:, k*ROW:k*ROW + 128],
                            G2[:, k*ROW + 128:k*ROW + 129].to_broadcast(
                                [128, 128]), MULT)
                    nc.vector.tensor_tensor(
                        gD[:, 0:128], gD[:, 0:128],
                        gD[:, 128:129].to_broadcast([128, 128]), MULT)
                    PT = psT.tile([128, 128], F32, tag="PT", name="PT")
                    for k in range(Kt):
                        nc.tensor.matmul(PT[:], lhsT=G2[:, k*ROW:k*ROW + 128],
                                         rhs=ident[:], start=(k == 0),
                                         stop=False)
                    nc.tensor.matmul(PT[:], lhsT=gD[:, 0:128], rhs=ident[:],
                                     start=False, stop=True)
                    aggT = stg.tile([128, 128], F32, tag="aggT", name="aggT")
                    nc.vector.tensor_copy(aggT[:], PT[:])
                    PO = psO.tile([128, NCLASS], F32, tag="PO", name="PO")
                    nc.tensor.matmul(PO[:], lhsT=aggT[:], rhs=w2t[:],
                                     start=True, stop=True)
                    o2 = stg.tile([128, NCLASS], F32, tag="o2", name="o2")
                    nc.vector.tensor_tensor(
                        o2[:], PO[:], rcp2[:].to_broadcast([128, NCLASS]),
                        MULT)
                    nc.vector.tensor_tensor(o2[:], o2[:], C["b2F"][:], ADD)
                    nc.sync.dma_start(out=out2[t*128:(t+1)*128, :], in_=o2[:])
    nc.compile()
    return nc


# ----------------------------------------------------------------- runner

def _device_forward(inputs):
    from concourse import bass_utils
    x = np.asarray(inputs["x"], np.float32)
    ei = np.asarray(inputs["edge_index"])
    offs, offs2, xT, meta = _prep_host(x, ei)
    wd = _prep_weights(inputs["W1"], inputs["att_src1"], inputs["att_dst1"],
                       inputs["b1"], inputs["W2"], inputs["att_src2"],
                       inputs["att_dst2"], inputs["b2"])
    if "nc" not in _CACHE:
        _CACHE["nc"] = _build(meta)
    nc = _CACHE["nc"]
    in_maps = [dict(xT=xT[c], offs=offs[c], offs2=offs2[c], **wd)
               for c in range(N_CORES)]
    res = None
    for attempt in range(3):
        try:
            res = bass_utils.run_bass_kernel_spmd(
                nc, in_maps, core_ids=list(range(N_CORES)))
            break
        except Exception as e:
            sys.stderr.write(f"[kernel] exec attempt {attempt} failed "
                             f"({e!r}); retrying\n")
            import time as _time
            _time.sleep(2.0)
    if res is None:
        res = bass_utils.run_bass_kernel_spmd(
            nc, in_maps, core_ids=list(range(N_CORES)))
    npc, N = meta["npc"], meta["N"]
    out_full = np.zeros((meta["npad"], NCLASS), np.float32)
    for c in range(N_CORES):
        out_full[meta["node_of_row"][c * npc:(c + 1) * npc]] = \
            res.results[c]["out2"]
    return out_full[:N]


def kernel(**inputs):
    for attempt in range(2):
        try:
            out = _device_forward(inputs)
            if not np.all(np.isfinite(out)):
                raise RuntimeError("non-finite device output")
            return out
        except Exception as e:
            sys.stderr.write(f"[kernel] device attempt {attempt} failed "
                             f"({e!r})\n")
    sys.stderr.write("[kernel] device path failed; numpy fallback\n")
    return _np_forward(
        np.asarray(inputs["x"], np.float32), inputs["edge_index"],
        inputs["W1"], inputs["att_src1"], inputs["att_dst1"], inputs["b1"],
        inputs["W2"], inputs["att_src2"], inputs["att_dst2"], inputs["b2"])
